# revision 8
# baseline (speedup 1.0000x reference)
"""Trainium2 Bass kernel for a Llama decoder layer (nn_MixedLlamaDecoderLayer_732).

Strategy (8-core tensor parallel, all column-parallel / all-gather based):
  - raw hidden^T replicated (bf16) to every core; ln1_w folded into Wq/Wk/Wv
    on host; per-token rsqrt factors computed locally from the core's token
    slice and exchanged via a tiny AllGather, then folded into the RoPE / V
    epilogues (exact same math as normalizing x first).
  - QKV + RoPE + causal attention head-sharded (4 Q heads / 1 KV head per
    core); attention output AllGathered per batch (4 chunks) so the
    collective hides under the next batch's compute.
  - o_proj column-parallel -> h1 column shard (fp32); ln2 stats via small
    per-pair AllReduces (4 chunks); ln2_w folded into gate/up weights;
    h2 column shard AllGathered in 4 chunks.
  - gate/up FF-sharded -> g^T AllGathered in 8 chunks (2 ff-halves x 4
    token pairs) -> down column-parallel over the gathered (reordered) g.
  - output = column shard of (h1 + mlp)^T, assembled + transposed on host.

All activations on-device are feature-major ("transposed": [features,
tokens]) so every matmul contraction dim lands on SBUF partitions.
Collectives are chunked and issued inside the producing loops so the
CC core / SDMA work overlaps PE compute instead of serializing stages.
"""

import os
import sys
from contextlib import ExitStack

os.environ.setdefault("JAX_PLATFORMS", "cpu")
if "/opt/trn_rl_repo" not in sys.path:
    sys.path.insert(0, "/opt/trn_rl_repo")

import numpy as np
import ml_dtypes

import concourse.bass as bass
import concourse.bacc as bacc
import concourse.tile as tile
from concourse import mybir

BF16 = mybir.dt.bfloat16
F32 = mybir.dt.float32
AF = mybir.ActivationFunctionType
ALU = mybir.AluOpType

NCORES = 8
B, S, HID = 4, 1024, 4096
T = B * S                      # 4096 tokens
NH, NKV, HD = 32, 8, 128
FF = 11008
EPS = 1e-6
THETA = 10000.0

QC = NH * HD // NCORES         # 512 q-cols per core (4 heads)
KC = HD                        # 128 kv-cols per core (1 kv head)
FFC = FF // NCORES             # 1376 ff per core
OC = HID // NCORES             # 512 out-cols per core
TS = T // NCORES               # 512 tokens per core
NHT = HID // 128               # 32 hid tiles
NTT = T // 512                 # 8 token tiles of 512
NPAIR = NTT // 2               # 4 token-tile pairs (= batches)
SCALE = 1.0 / float(np.sqrt(HD))

# ff tile sizes within a core's 1376 columns: 10x128 + 96, split in halves
FF_TILES = [(i * 128, 128) for i in range(10)] + [(1280, 96)]
HALVES = [FF_TILES[:6], FF_TILES[6:]]      # 768 rows | 608 rows per core
HSZ = [768, 608]
GROWS = [HSZ[0] * NCORES, HSZ[1] * NCORES]  # 6144, 4864 gathered rows
GT0 = GROWS[0] // 128                       # 48 gathered f-tiles (half 0)
GT1 = GROWS[1] // 128                       # 38 gathered f-tiles (half 1)
NFT = GT0 + GT1                             # 86


def build_nc():
    nc = bacc.Bacc("TRN2", target_bir_lowering=False, debug=False,
                   num_devices=NCORES)
    d = {}
    ein = lambda n, s, t: nc.dram_tensor(n, s, t, kind="ExternalInput")
    d["hid_c"] = ein("hid_c", [TS, HID], F32)        # own token slice (for r)
    d["hidT_f"] = ein("hidT_f", [HID, T], BF16)      # replicated raw hidden^T
    d["hidT_c"] = ein("hidT_c", [OC, T], F32)        # own hid-col slice (resid)
    d["wq_t"] = ein("wq_t", [128, NHT * QC], BF16)   # ln1-folded
    d["wk_t"] = ein("wk_t", [128, NHT * KC], BF16)
    d["wv_t"] = ein("wv_t", [128, NHT * KC], BF16)
    d["wo_t"] = ein("wo_t", [128, NHT * OC], BF16)
    d["gate_t"] = ein("gate_t", [128, NHT * FFC], BF16)  # ln2-folded
    d["up_t"] = ein("up_t", [128, NHT * FFC], BF16)
    d["down_t"] = ein("down_t", [128, NFT * OC], BF16)   # rows reordered
    d["cosT"] = ein("cosT", [128, T], BF16)
    d["sinS"] = ein("sinS", [128, T], BF16)          # sign-folded sin
    d["maskT"] = ein("maskT", [128, (S // 128) * S], BF16)
    d["ident"] = ein("ident", [128, 128], BF16)
    d["ones128"] = ein("ones128", [128, 1], BF16)
    d["ones1"] = ein("ones1", [1, 128], BF16)
    out_c = nc.dram_tensor("out_c", [OC, T], F32, kind="ExternalOutput")

    # ---- internal DRAM (collective bounce buffers) ----
    r_in = nc.dram_tensor("r_in", [1, TS], F32)
    ag_r = nc.dram_tensor("ag_r", [NCORES, TS], F32, addr_space="Shared")
    ag_at_in = [nc.dram_tensor(f"ag_at_in{b}", [QC, S], BF16)
                for b in range(B)]
    ag_at = [nc.dram_tensor(f"ag_at{b}", [NH * HD, S], BF16,
                            addr_space="Shared") for b in range(B)]
    ar_sq_in = [nc.dram_tensor(f"ar_sq_in{p}", [1, 1024], F32)
                for p in range(NPAIR)]
    ar_sq = [nc.dram_tensor(f"ar_sq{p}", [1, 1024], F32, addr_space="Shared")
             for p in range(NPAIR)]
    ag_h2_in = [nc.dram_tensor(f"ag_h2_in{p}", [OC, 1024], BF16)
                for p in range(NPAIR)]
    ag_h2 = [nc.dram_tensor(f"ag_h2{p}", [HID, 1024], BF16,
                            addr_space="Shared") for p in range(NPAIR)]
    ag_g_in = [[nc.dram_tensor(f"ag_g_in{h}_{p}", [HSZ[h], 1024], BF16)
                for p in range(NPAIR)] for h in range(2)]
    ag_g = [[nc.dram_tensor(f"ag_g{h}_{p}", [GROWS[h], 1024], BF16,
                            addr_space="Shared") for p in range(NPAIR)]
            for h in range(2)]
    h1_spill = nc.dram_tensor("h1_spill", [OC, T], F32)

    RG = [list(range(NCORES))]

    def agather(ins, outs):
        nc.gpsimd.collective_compute("AllGather", ALU.bypass,
                                     replica_groups=RG, ins=[ins], outs=[outs])

    with tile.TileContext(nc) as tc:
        with tc.tile_pool(name="consts", bufs=1) as consts:
            ident = consts.tile([128, 128], BF16)
            nc.sync.dma_start(ident[:], d["ident"][:])
            ones128 = consts.tile([128, 1], BF16)
            nc.sync.dma_start(ones128[:], d["ones128"][:])
            ones1 = consts.tile([1, 128], BF16)
            nc.sync.dma_start(ones1[:], d["ones1"][:])
            eps128 = consts.tile([128, 1], F32)
            nc.gpsimd.memset(eps128[:], EPS)

            # ===== Stage A-lite: rms factors for own tokens + tiny AG =====
            with (
                tc.tile_pool(name="a_in", bufs=2) as a_in,
                tc.tile_pool(name="a_tmp", bufs=2) as a_tmp,
                tc.tile_pool(name="a_keep", bufs=1) as a_keep,
            ):
                rstage = a_keep.tile([128, TS // 128], F32, tag="rstage")
                for p4 in range(TS // 128):
                    ht = a_in.tile([128, HID], F32, tag="hin", name=f"ht_{p4}")
                    nc.sync.dma_start(ht[:], d["hid_c"][p4 * 128:(p4 + 1) * 128, :])
                    sq = a_tmp.tile([128, HID], F32, tag="sq", name="sq")
                    ssq = a_tmp.tile([128, 1], F32, tag="ssq", name="ssq")
                    nc.scalar.activation(sq[:], ht[:], AF.Square, accum_out=ssq[:])
                    st = a_tmp.tile([128, 1], F32, tag="st", name="st")
                    nc.scalar.activation(st[:], ssq[:], AF.Sqrt,
                                         scale=1.0 / HID, bias=eps128[:, 0:1])
                    nc.vector.reciprocal(rstage[:, p4:p4 + 1], st[:])
                nc.sync.dma_start(
                    r_in[:].rearrange("o (b p) -> (o p) b", p=128), rstage[:])
            agather(r_in[:], ag_r[:])

            # ===== Stage B: QKV (raw x) + r-folded RoPE (head shard) =====
            bc_es = ExitStack()
            bc_keep = bc_es.enter_context(tc.tile_pool(name="bc_keep", bufs=1))
            qT = bc_keep.tile([128, 4 * T], BF16, tag="qT")
            kT = bc_keep.tile([128, T], BF16, tag="kT")
            vS = bc_keep.tile([128, T], BF16, tag="vS")
            cosT = bc_keep.tile([128, T], BF16, tag="cosT")
            nc.sync.dma_start(cosT[:], d["cosT"][:])
            sinS = bc_keep.tile([128, T], BF16, tag="sinS")
            nc.sync.dma_start(sinS[:], d["sinS"][:])

            with (
                tc.tile_pool(name="b_w", bufs=1) as b_w,
                tc.tile_pool(name="b_x", bufs=4) as b_x,
                tc.tile_pool(name="b_tmp", bufs=2) as b_tmp,
                tc.tile_pool(name="b_rb", bufs=2) as b_rb,
                tc.tile_pool(name="b_ps", bufs=1, space="PSUM") as b_ps,
                tc.tile_pool(name="b_psr", bufs=1, space="PSUM") as b_psr,
            ):
                wq = b_w.tile([128, NHT * QC], BF16)
                nc.sync.dma_start(wq[:], d["wq_t"][:])
                wk = b_w.tile([128, NHT * KC], BF16)
                nc.sync.dma_start(wk[:], d["wk_t"][:])
                wv = b_w.tile([128, NHT * KC], BF16)
                nc.sync.dma_start(wv[:], d["wv_t"][:])

                def rope_fold(dst, dst_off, ps, cs_off, rbt):
                    """dst[:, dst_off:+512] = rope(ps) * r  (r broadcast)."""
                    c_lo = cosT[0:64, cs_off:cs_off + 512]
                    c_hi = cosT[64:128, cs_off:cs_off + 512]
                    s_lo = sinS[0:64, cs_off:cs_off + 512]
                    s_hi = sinS[64:128, cs_off:cs_off + 512]
                    t1 = b_tmp.tile([128, 512], F32, tag="ro1", name="ro1")
                    nc.vector.tensor_mul(t1[0:64, :], ps[64:128, :], s_lo)
                    nc.vector.tensor_mul(t1[64:128, :], ps[0:64, :], s_hi)
                    t2 = b_tmp.tile([128, 512], F32, tag="ro2", name="ro2")
                    nc.vector.tensor_mul(t2[0:64, :], ps[0:64, :], c_lo)
                    nc.vector.tensor_mul(t2[64:128, :], ps[64:128, :], c_hi)
                    t3 = b_tmp.tile([128, 512], F32, tag="ro3", name="ro3")
                    nc.vector.tensor_add(t3[:], t1[:], t2[:])
                    nc.vector.tensor_mul(dst[:, dst_off:dst_off + 512],
                                         t3[:], rbt[:])

                for tt in range(NTT):
                    # broadcast r for this token tile: ag_r row tt
                    rrow = b_tmp.tile([1, 512], F32, tag="rrow", name=f"rr_{tt}")
                    nc.sync.dma_start(rrow[:], ag_r[tt:tt + 1, :])
                    rrowb = b_tmp.tile([1, 512], BF16, tag="rrowb", name="rrb")
                    nc.vector.tensor_copy(rrowb[:], rrow[:])
                    ps_rb = b_psr.tile([128, 512], F32, tag="tpv",
                                       name=f"ps_rb_{tt}", bufs=2)
                    nc.tensor.matmul(ps_rb[:], ones1[:], rrowb[:],
                                     start=True, stop=True)
                    rbt = b_rb.tile([128, 512], BF16, tag="rbt",
                                    name=f"rbt_{tt}")
                    nc.vector.tensor_copy(rbt[:], ps_rb[:])

                    psq = [b_ps.tile([128, 512], F32, tag=f"psq{i}",
                                     name=f"psq{i}_{tt}") for i in range(4)]
                    psk = b_ps.tile([128, 512], F32, tag="psk", name=f"psk_{tt}")
                    psv = b_ps.tile([128, 512], F32, tag="psv", name=f"psv_{tt}")
                    for h4 in range(NHT // 4):
                        xt4 = b_x.tile([128, 4 * 512], BF16, tag="xt",
                                       name=f"xt_{tt}_{h4}")
                        nc.sync.dma_start(
                            xt4[:].rearrange("p (a t) -> p a t", t=512),
                            d["hidT_f"][h4 * 512:(h4 + 1) * 512,
                                        tt * 512:(tt + 1) * 512]
                            .rearrange("(a p) t -> p a t", p=128))
                        for a in range(4):
                            h = h4 * 4 + a
                            xt = xt4[:, a * 512:(a + 1) * 512]
                            for qc in range(4):
                                nc.tensor.matmul(
                                    psq[qc][:],
                                    wq[:, h * QC + qc * 128: h * QC + (qc + 1) * 128],
                                    xt, start=(h == 0), stop=(h == NHT - 1))
                            nc.tensor.matmul(psk[:], wk[:, h * KC:(h + 1) * KC],
                                             xt, start=(h == 0), stop=(h == NHT - 1))
                            nc.tensor.matmul(psv[:], wv[:, h * KC:(h + 1) * KC],
                                             xt, start=(h == 0), stop=(h == NHT - 1))
                    for qc in range(4):
                        rope_fold(qT, qc * T + tt * 512, psq[qc], tt * 512, rbt)
                    rope_fold(kT, tt * 512, psk, tt * 512, rbt)
                    # V^T * r, then transpose 128-blocks into token-major vS
                    vtmp = b_tmp.tile([128, 512], BF16, tag="vtmp",
                                      name=f"vtmp_{tt}")
                    nc.vector.tensor_mul(vtmp[:], psv[:], rbt[:])
                    for s4 in range(4):
                        pvt = b_psr.tile([128, 128], BF16, tag="tpv",
                                         name=f"tpv_{tt}_{s4}", bufs=2)
                        nc.tensor.transpose(pvt[:], vtmp[:, s4 * 128:(s4 + 1) * 128],
                                            ident[:])
                        nc.vector.tensor_copy(
                            vS[:, (tt * 4 + s4) * 128:(tt * 4 + s4 + 1) * 128],
                            pvt[:])

            # ===== Stage C: causal attention, AG chunk per batch =====
            with (
                tc.tile_pool(name="c_pt", bufs=2) as c_pt,
                tc.tile_pool(name="c_keep", bufs=1) as c_keep,
                tc.tile_pool(name="c_tmp", bufs=4) as c_tmp,
                tc.tile_pool(name="c_ps", bufs=2, space="PSUM") as c_ps,
                tc.tile_pool(name="c_psd", bufs=2, space="PSUM") as c_psd,
            ):
                maskT = c_keep.tile([128, (S // 128) * S], BF16, tag="maskT")
                nc.sync.dma_start(maskT[:], d["maskT"][:])
                NKT = S // 128  # 8 k tiles per batch
                for b in range(B):
                    for h in range(4):
                        pt = c_pt.tile([128, NKT * S], BF16, tag="pt",
                                       name=f"pt_{b}_{h}")
                        qoff = h * T + b * S
                        for kt in range(NKT):
                            for q2 in range(2):
                                if kt * 128 >= (q2 + 1) * 512:
                                    continue
                                pss = c_ps.tile([128, 512], F32, tag="pss",
                                                name=f"pss_{b}_{h}_{kt}_{q2}")
                                nc.tensor.matmul(
                                    pss[:],
                                    kT[:, b * S + kt * 128: b * S + (kt + 1) * 128],
                                    qT[:, qoff + q2 * 512: qoff + (q2 + 1) * 512],
                                    start=True, stop=True)
                                po = kt * S + q2 * 512
                                nc.vector.scalar_tensor_tensor(
                                    pt[:, po:po + 512], pss[:], SCALE,
                                    maskT[:, kt * S + q2 * 512: kt * S + (q2 + 1) * 512],
                                    op0=ALU.mult, op1=ALU.add)
                                nc.scalar.activation(pt[:, po:po + 512],
                                                     pt[:, po:po + 512], AF.Exp)
                        for q2 in range(2):
                            nk = min(NKT, (q2 + 1) * 4)
                            psd = c_psd.tile([1, 512], F32, tag="psd",
                                             name=f"psd_{b}_{h}_{q2}")
                            for kt in range(nk):
                                nc.tensor.matmul(
                                    psd[:], ones128[:],
                                    pt[:, kt * S + q2 * 512: kt * S + (q2 + 1) * 512],
                                    start=(kt == 0), stop=(kt == nk - 1))
                            dnr = c_tmp.tile([1, 512], F32, tag="dnr", name="dnr")
                            nc.vector.reciprocal(dnr[:], psd[:])
                            dnb = c_tmp.tile([1, 512], BF16, tag="dnb", name="dnb")
                            nc.vector.tensor_copy(dnb[:], dnr[:])
                            psr = c_psd.tile([128, 512], F32, tag="psr",
                                             name=f"psr_{b}_{h}_{q2}")
                            nc.tensor.matmul(psr[:], ones1[:], dnb[:],
                                             start=True, stop=True)
                            rb = c_tmp.tile([128, 512], BF16, tag="rb", name="rb")
                            nc.vector.tensor_copy(rb[:], psr[:])
                            psa = c_ps.tile([128, 512], F32, tag="psa",
                                            name=f"psa_{b}_{h}_{q2}")
                            for kt in range(nk):
                                nc.tensor.matmul(
                                    psa[:],
                                    vS[:, (b * 8 + kt) * 128:(b * 8 + kt + 1) * 128],
                                    pt[:, kt * S + q2 * 512: kt * S + (q2 + 1) * 512],
                                    start=(kt == 0), stop=(kt == nk - 1))
                            ao = c_tmp.tile([128, 512], BF16, tag="ao", name="ao")
                            nc.vector.tensor_mul(ao[:], psa[:], rb[:])
                            nc.sync.dma_start(
                                ag_at_in[b][h * 128:(h + 1) * 128,
                                            q2 * 512:(q2 + 1) * 512],
                                ao[:])
                    agather(ag_at_in[b][:], ag_at[b][:])
            bc_es.close()

            # E-weight streaming pool opened early: quarter-sized slots,
            # double-buffered, so gate/up quarter loads hide under D and E.
            de_es = ExitStack()
            e_w = de_es.enter_context(tc.tile_pool(name="e_w", bufs=2))
            QUARTERS = [FF_TILES[0:2], FF_TILES[2:4], FF_TILES[4:6],
                        FF_TILES[6:8], FF_TILES[8:10], FF_TILES[10:11]]
            QSZ = [sum(fw for _, fw in q) for q in QUARTERS]  # 256*5, 96
            QSLOT = max(QSZ)
            NQ = len(QUARTERS)

            def load_qw(src, qi, nm):
                q_lo = QUARTERS[qi][0][0]
                t = e_w.tile([128, NHT * QSLOT], BF16, tag=nm,
                             name=f"{nm}_{qi}")
                nc.sync.dma_start(
                    t[:, 0:NHT * QSZ[qi]].rearrange("p (h f) -> p h f",
                                                    f=QSZ[qi]),
                    src.rearrange("p (h f) -> p h f", f=FFC)
                    [:, :, q_lo:q_lo + QSZ[qi]])
                return t

            gw0 = load_qw(d["gate_t"], 0, "gw")
            uw0 = load_qw(d["up_t"], 0, "uw")

            # ===== Stage D: o_proj col-parallel + residual + chunked ln2 =====
            with (
                tc.tile_pool(name="d_w", bufs=1) as d_w,
                tc.tile_pool(name="d_x", bufs=3) as d_x,
                tc.tile_pool(name="d_tmp", bufs=2) as d_tmp,
                tc.tile_pool(name="d_h1", bufs=2) as d_h1,
                tc.tile_pool(name="d_rb", bufs=1) as d_rb,
                tc.tile_pool(name="d_ps", bufs=1, space="PSUM") as d_ps,
                tc.tile_pool(name="d_pss", bufs=2, space="PSUM") as d_pss,
            ):
                wo = d_w.tile([128, NHT * OC], BF16)
                nc.sync.dma_start(wo[:], d["wo_t"][:])
                for p in range(NPAIR):
                    h1bf = d_h1.tile([128, 4 * 1024], BF16, tag="h1bf",
                                     name=f"h1bf_{p}")
                    sqst = d_h1.tile([1, 1024], F32, tag="sqst",
                                     name=f"sqst_{p}")
                    for ti in range(2):
                        tt = 2 * p + ti
                        pso = [d_ps.tile([128, 512], F32, tag=f"pso{i}",
                                         name=f"pso{i}_{tt}") for i in range(4)]
                        for a4 in range(NHT // 4):
                            at4 = d_x.tile([128, 4 * 512], BF16, tag="at",
                                           name=f"at_{tt}_{a4}")
                            nc.sync.dma_start(
                                at4[:].rearrange("p (a t) -> p a t", t=512),
                                ag_at[p][a4 * 512:(a4 + 1) * 512,
                                         ti * 512:(ti + 1) * 512]
                                .rearrange("(a p) t -> p a t", p=128))
                            for a in range(4):
                                ac = a4 * 4 + a
                                at = at4[:, a * 512:(a + 1) * 512]
                                for oc in range(4):
                                    nc.tensor.matmul(
                                        pso[oc][:],
                                        wo[:, ac * OC + oc * 128: ac * OC + (oc + 1) * 128],
                                        at, start=(ac == 0), stop=(ac == NHT - 1))
                        ps_ssq = d_pss.tile([1, 512], F32, tag="ps_ssq",
                                            name=f"ps_ssq_{tt}")
                        hT4 = d_tmp.tile([128, 4 * 512], F32, tag="hTt",
                                         name="hT4")
                        nc.sync.dma_start(
                            hT4[:].rearrange("p (a t) -> p a t", t=512),
                            d["hidT_c"][:, tt * 512:(tt + 1) * 512]
                            .rearrange("(a p) t -> p a t", p=128))
                        h1f4 = d_tmp.tile([128, 4 * 512], F32, tag="h1f",
                                          name="h1f4")
                        for oc in range(4):
                            h1f = h1f4[:, oc * 512:(oc + 1) * 512]
                            nc.vector.tensor_add(h1f, pso[oc][:],
                                                 hT4[:, oc * 512:(oc + 1) * 512])
                            hb_off = oc * 1024 + ti * 512
                            nc.vector.tensor_copy(h1bf[:, hb_off:hb_off + 512],
                                                  h1f)
                            h1sq = d_tmp.tile([128, 512], BF16, tag="h1sq",
                                              name="h1sq")
                            nc.scalar.activation(h1sq[:], h1f, AF.Square)
                            nc.tensor.matmul(ps_ssq[:], ones128[:], h1sq[:],
                                             start=(oc == 0), stop=(oc == 3))
                        nc.sync.dma_start(
                            h1_spill[:, tt * 512:(tt + 1) * 512]
                            .rearrange("(a p) t -> p a t", p=128),
                            h1f4[:].rearrange("p (a t) -> p a t", t=512))
                        nc.vector.tensor_copy(sqst[:, ti * 512:(ti + 1) * 512],
                                              ps_ssq[:])
                    nc.sync.dma_start(ar_sq_in[p][:], sqst[:])
                    nc.gpsimd.collective_compute(
                        "AllReduce", ALU.add, replica_groups=RG,
                        ins=[ar_sq_in[p][:]], outs=[ar_sq[p][:]])
                    ssqF = d_tmp.tile([1, 1024], F32, tag="u1", name=f"ssqF_{p}")
                    nc.sync.dma_start(ssqF[:], ar_sq[p][:])
                    sF = d_tmp.tile([1, 1024], F32, tag="u0", name="sF")
                    nc.scalar.activation(sF[:], ssqF[:], AF.Sqrt,
                                         scale=1.0 / HID, bias=eps128[0:1, 0:1])
                    rF = d_tmp.tile([1, 1024], F32, tag="u1b", name="rF")
                    nc.vector.reciprocal(rF[:], sF[:])
                    rFb = d_tmp.tile([1, 1024], BF16, tag="rFb", name="rFb")
                    nc.vector.tensor_copy(rFb[:], rF[:])
                    rb2 = d_rb.tile([128, 1024], BF16, tag="rb2",
                                    name=f"rb2_{p}")
                    for ti in range(2):
                        psrb = d_pss.tile([128, 512], F32, tag="psrb",
                                          name=f"psrb_{p}_{ti}")
                        nc.tensor.matmul(psrb[:], ones1[:],
                                         rFb[:, ti * 512:(ti + 1) * 512],
                                         start=True, stop=True)
                        nc.vector.tensor_copy(rb2[:, ti * 512:(ti + 1) * 512],
                                              psrb[:])
                    for ti in range(2):
                        h2t4 = d_tmp.tile([128, 4 * 512], BF16, tag="h2t",
                                          name=f"h2t4_{p}_{ti}")
                        for oc in range(4):
                            nc.vector.tensor_mul(
                                h2t4[:, oc * 512:(oc + 1) * 512],
                                h1bf[:, oc * 1024 + ti * 512: oc * 1024 + (ti + 1) * 512],
                                rb2[:, ti * 512:(ti + 1) * 512])
                        nc.sync.dma_start(
                            ag_h2_in[p][:, ti * 512:(ti + 1) * 512]
                            .rearrange("(a p) t -> p a t", p=128),
                            h2t4[:].rearrange("p (a t) -> p a t", t=512))
                    agather(ag_h2_in[p][:], ag_h2[p][:])

            # ===== Stage E: gate/up (ff shard), quarter-streamed weights ====
            with (
                tc.tile_pool(name="e_x", bufs=2) as e_x,
                tc.tile_pool(name="e_tmp", bufs=3) as e_tmp,
                tc.tile_pool(name="e_ps", bufs=2, space="PSUM") as e_ps,
            ):
                for qi in range(NQ):
                    if qi == 0:
                        gw, uw = gw0, uw0
                    else:
                        gw = load_qw(d["gate_t"], qi, "gw")
                        uw = load_qw(d["up_t"], qi, "uw")
                    q_lo = QUARTERS[qi][0][0]
                    hi = 0 if q_lo < HSZ[0] else 1
                    h_lo = 0 if hi == 0 else HSZ[0]
                    for p in range(NPAIR):
                        for ti in range(2):
                            tt = 2 * p + ti
                            h2full = e_x.tile([128, NHT * 512], BF16,
                                              tag="h2full",
                                              name=f"h2full_{qi}_{tt}")
                            for h4 in range(NHT // 4):
                                nc.sync.dma_start(
                                    h2full[:, h4 * 2048:(h4 + 1) * 2048]
                                    .rearrange("p (a t) -> p a t", t=512),
                                    ag_h2[p][h4 * 512:(h4 + 1) * 512,
                                             ti * 512:(ti + 1) * 512]
                                    .rearrange("(a p) t -> p a t", p=128))
                            for (fo, fw) in QUARTERS[qi]:
                                psg = e_ps.tile([128, 512], F32, tag="psg",
                                                name=f"psg_{tt}_{fo}")
                                psu = e_ps.tile([128, 512], F32, tag="psu",
                                                name=f"psu_{tt}_{fo}")
                                lo = fo - q_lo
                                for h in range(NHT):
                                    nc.tensor.matmul(
                                        psg[0:fw, :],
                                        gw[:, h * QSZ[qi] + lo: h * QSZ[qi] + lo + fw],
                                        h2full[:, h * 512:(h + 1) * 512],
                                        start=(h == 0), stop=(h == NHT - 1))
                                for h in range(NHT):
                                    nc.tensor.matmul(
                                        psu[0:fw, :],
                                        uw[:, h * QSZ[qi] + lo: h * QSZ[qi] + lo + fw],
                                        h2full[:, h * 512:(h + 1) * 512],
                                        start=(h == 0), stop=(h == NHT - 1))
                                gsig = e_tmp.tile([128, 512], F32, tag="gsig",
                                                  name="gsig")
                                nc.scalar.activation(gsig[0:fw, :], psg[0:fw, :],
                                                     AF.Sigmoid)
                                gsil = e_tmp.tile([128, 512], BF16, tag="gsil",
                                                  name="gsil")
                                nc.vector.tensor_mul(gsil[0:fw, :], gsig[0:fw, :],
                                                     psg[0:fw, :])
                                gt = e_tmp.tile([128, 512], BF16, tag="gt",
                                                name="gt")
                                nc.vector.tensor_mul(gt[0:fw, :], gsil[0:fw, :],
                                                     psu[0:fw, :])
                                nc.sync.dma_start(
                                    ag_g_in[hi][p][fo - h_lo:fo - h_lo + fw,
                                                   ti * 512:(ti + 1) * 512],
                                    gt[0:fw, :])
                        if qi == 2:
                            agather(ag_g_in[0][p][:], ag_g[0][p][:])
                        elif qi == NQ - 1:
                            agather(ag_g_in[1][p][:], ag_g[1][p][:])
            de_es.close()

            # ===== Stage F: down col-parallel over gathered g + residual ====
            with (
                tc.tile_pool(name="f_w", bufs=1) as f_w,
                tc.tile_pool(name="f_x", bufs=4) as f_x,
                tc.tile_pool(name="f_tmp", bufs=2) as f_tmp,
                tc.tile_pool(name="f_ps", bufs=2, space="PSUM") as f_ps,
            ):
                dw = f_w.tile([128, NFT * OC], BF16)
                nc.sync.dma_start(dw[:], d["down_t"][:])
                for tt in range(NTT):
                    p, ti = tt // 2, tt % 2
                    psd = [f_ps.tile([128, 512], F32, tag=f"psd{oc}",
                                     name=f"psd{oc}_{tt}")
                           for oc in range(4)]
                    f_base = 0
                    for hi in range(2):
                        nht = GROWS[hi] // 128
                        for f4 in range(0, nht, 4):
                            nf = min(4, nht - f4)
                            gx4 = f_x.tile([128, 4 * 512], BF16, tag="gx",
                                           name=f"gx_{tt}_{hi}_{f4}")
                            nc.sync.dma_start(
                                gx4[:, 0:nf * 512]
                                .rearrange("p (a t) -> p a t", t=512),
                                ag_g[hi][p][f4 * 128:(f4 + nf) * 128,
                                            ti * 512:(ti + 1) * 512]
                                .rearrange("(a p) t -> p a t", p=128))
                            for a in range(nf):
                                f = f_base + f4 + a
                                gx = gx4[:, a * 512:(a + 1) * 512]
                                for oc in range(4):
                                    nc.tensor.matmul(
                                        psd[oc][:],
                                        dw[:, f * OC + oc * 128: f * OC + (oc + 1) * 128],
                                        gx, start=(f == 0), stop=(f == NFT - 1))
                        f_base += nht
                    h1t4 = f_tmp.tile([128, 4 * 512], F32, tag="h1t",
                                      name="h1t4")
                    nc.sync.dma_start(
                        h1t4[:].rearrange("p (a t) -> p a t", t=512),
                        h1_spill[:, tt * 512:(tt + 1) * 512]
                        .rearrange("(a p) t -> p a t", p=128))
                    ot4 = f_tmp.tile([128, 4 * 512], F32, tag="ot", name="ot4")
                    for oc in range(4):
                        nc.vector.tensor_add(ot4[:, oc * 512:(oc + 1) * 512],
                                             psd[oc][:],
                                             h1t4[:, oc * 512:(oc + 1) * 512])
                    nc.sync.dma_start(
                        out_c[:, tt * 512:(tt + 1) * 512]
                        .rearrange("(a p) t -> p a t", p=128),
                        ot4[:].rearrange("p (a t) -> p a t", t=512))

    nc.compile()
    return nc


def host_prep(inputs):
    """Build the 8 per-core input maps from full-size inputs."""
    bf = ml_dtypes.bfloat16
    hs = np.asarray(inputs["hidden_states"], np.float32)
    pos = np.asarray(inputs["position_ids"]).astype(np.int64).reshape(-1)  # [T]
    mask = np.asarray(inputs["attn_mask"], np.float32).reshape(S, S)
    ln1 = np.asarray(inputs["ln1_w"], np.float32)
    ln2 = np.asarray(inputs["ln2_w"], np.float32)
    # fold rmsnorm gains into the consuming weights (exact rewrite)
    Wq = ln1[:, None] * np.asarray(inputs["Wq"], np.float32)
    Wk = ln1[:, None] * np.asarray(inputs["Wk"], np.float32)
    Wv = ln1[:, None] * np.asarray(inputs["Wv"], np.float32)
    Wo = np.asarray(inputs["Wo"], np.float32)
    wg = ln2[:, None] * np.asarray(inputs["w_gate"], np.float32)
    wu = ln2[:, None] * np.asarray(inputs["w_up"], np.float32)
    wd = np.asarray(inputs["w_down"], np.float32)

    hsT = np.ascontiguousarray(hs.T)
    hsT_bf = hsT.astype(bf)

    invf = 1.0 / (THETA ** (np.arange(0, HD, 2, dtype=np.float32) / HD))  # [64]
    ang = pos[None, :].astype(np.float32) * np.concatenate([invf, invf])[:, None]
    cosT = np.cos(ang)                    # [128, T]
    sinT = np.sin(ang)
    sinS = sinT.copy()
    sinS[:64] *= -1.0

    maskT = np.ascontiguousarray(mask.T)  # [k, q]
    maskT_b = maskT.reshape(S // 128, 128, S).transpose(1, 0, 2).reshape(128, -1)

    def tile_w(w, cols):
        # [HID, cols] -> [128, NHT*cols] with [:, h*cols:+cols] = w[128h:+128, :]
        return np.ascontiguousarray(
            w.reshape(NHT, 128, cols).transpose(1, 0, 2).reshape(128, NHT * cols)
        ).astype(bf)

    # down rows reordered to match the gathered-g layout:
    # half0: [core0 rows 0:768, core1 rows 0:768, ...], half1: [core0 768:1376,..]
    perm = np.concatenate(
        [np.arange(c * FFC, c * FFC + HSZ[0]) for c in range(NCORES)] +
        [np.arange(c * FFC + HSZ[0], (c + 1) * FFC) for c in range(NCORES)])
    wd_r = wd[perm]

    ident = np.eye(128, dtype=np.float32).astype(bf)
    ones128 = np.ones((128, 1), np.float32).astype(bf)
    ones1 = np.ones((1, 128), np.float32).astype(bf)

    in_maps = []
    for c in range(NCORES):
        qs, ks = c * QC, c * KC
        fs, os_ = c * FFC, c * OC
        wd_cols = np.ascontiguousarray(wd_r[:, os_:os_ + OC])  # [FF, OC]
        m = {
            "hid_c": np.ascontiguousarray(hs[c * TS:(c + 1) * TS]),
            "hidT_f": hsT_bf,
            "hidT_c": np.ascontiguousarray(hsT[os_:os_ + OC]),
            "wq_t": tile_w(np.ascontiguousarray(Wq[:, qs:qs + QC]), QC),
            "wk_t": tile_w(np.ascontiguousarray(Wk[:, ks:ks + KC]), KC),
            "wv_t": tile_w(np.ascontiguousarray(Wv[:, ks:ks + KC]), KC),
            "wo_t": tile_w(np.ascontiguousarray(Wo[:, os_:os_ + OC]), OC),
            "gate_t": tile_w(np.ascontiguousarray(wg[:, fs:fs + FFC]), FFC),
            "up_t": tile_w(np.ascontiguousarray(wu[:, fs:fs + FFC]), FFC),
            "down_t": np.ascontiguousarray(
                wd_cols.reshape(NFT, 128, OC).transpose(1, 0, 2)
                .reshape(128, NFT * OC)).astype(bf),
            "cosT": cosT.astype(bf),
            "sinS": sinS.astype(bf),
            "maskT": maskT_b.astype(bf),
            "ident": ident,
            "ones128": ones128,
            "ones1": ones1,
        }
        in_maps.append(m)
    return in_maps


_NC_CACHE = {}


def get_nc():
    if "nc" not in _NC_CACHE:
        _NC_CACHE["nc"] = build_nc()
    return _NC_CACHE["nc"]


def assemble(results):
    outT = np.concatenate([results[c]["out_c"] for c in range(NCORES)], axis=0)
    return np.ascontiguousarray(outT.T.astype(np.float32))


def _get_runner():
    """Build (once) a jitted SPMD callable over the 8 neuron cores."""
    if "runner" in _NC_CACHE:
        return _NC_CACHE["runner"]
    import jax
    from jax.sharding import Mesh, PartitionSpec, NamedSharding
    from jax.experimental.shard_map import shard_map
    from concourse import bass2jax, mybir as mb
    from concourse.bass2jax import _bass_exec_p, install_neuronx_cc_hook

    nc = get_nc()
    install_neuronx_cc_hook()
    in_names, out_names, out_avals, zero_outs = [], [], [], []
    partition_name = (nc.partition_id_tensor.name
                      if nc.partition_id_tensor else None)
    for alloc in nc.m.functions[0].allocations:
        if not isinstance(alloc, mb.MemoryLocationSet):
            continue
        name = alloc.memorylocations[0].name
        if alloc.kind == "ExternalInput":
            if name != partition_name:
                in_names.append(name)
        elif alloc.kind == "ExternalOutput":
            out_names.append(name)
            shape = tuple(alloc.tensor_shape)
            dtype = mb.dt.np(alloc.dtype)
            out_avals.append(jax.core.ShapedArray(shape, dtype))
            zero_outs.append(np.zeros(shape, dtype))
    n_params = len(in_names)
    n_outs = len(out_avals)
    all_in_names = list(in_names) + list(out_names)
    if partition_name is not None:
        all_in_names.append(partition_name)

    def _body(*args):
        operands = list(args)
        if partition_name is not None:
            operands.append(bass2jax.partition_id_tensor())
        outs = _bass_exec_p.bind(
            *operands,
            out_avals=tuple(out_avals),
            in_names=tuple(all_in_names),
            out_names=tuple(out_names),
            lowering_input_output_aliases=(),
            sim_require_finite=True,
            sim_require_nnan=True,
            nc=nc,
        )
        return tuple(outs)

    devices = jax.devices()[:NCORES]
    mesh = Mesh(np.asarray(devices), ("core",))
    donate = tuple(range(n_params, n_params + n_outs))
    sharded = jax.jit(
        shard_map(_body, mesh=mesh,
                  in_specs=(PartitionSpec("core"),) * (n_params + n_outs),
                  out_specs=(PartitionSpec("core"),) * n_outs,
                  check_rep=False),
        donate_argnums=donate, keep_unused=True)
    runner = {
        "jax": jax, "sharded": sharded, "in_names": in_names,
        "out_names": out_names, "out_avals": out_avals,
        "zero_outs": zero_outs, "mesh": mesh,
        "sharding": NamedSharding(mesh, PartitionSpec("core")),
    }
    _NC_CACHE["runner"] = runner
    return runner


def _run_hw(in_maps, bench_iters=0):
    r = _get_runner()
    jax = r["jax"]
    concat_in = [
        np.concatenate([np.asarray(in_maps[c][n]) for c in range(NCORES)],
                       axis=0) for n in r["in_names"]]
    concat_zeros = [np.zeros((NCORES * z.shape[0], *z.shape[1:]), z.dtype)
                    for z in r["zero_outs"]]
    din = [jax.device_put(a, r["sharding"]) for a in concat_in]
    out = r["sharded"](*din, *[jax.device_put(z, r["sharding"])
                               for z in concat_zeros])
    jax.block_until_ready(out)
    out_np = [np.asarray(o) for o in out]
    if bench_iters:
        import time
        import jax.numpy as jnp
        zshapes = [(NCORES * z.shape[0], *z.shape[1:]) for z in r["zero_outs"]]
        zdtypes = [z.dtype for z in r["zero_outs"]]
        zfn = jax.jit(
            lambda: tuple(jnp.zeros(s, d) for s, d in zip(zshapes, zdtypes)),
            out_shardings=tuple(r["sharding"] for _ in zshapes))

        def run_k(k):
            dzs = [zfn() for _ in range(k)]
            jax.block_until_ready(dzs)
            t0 = time.perf_counter()
            outs = [r["sharded"](*din, *dzs[i]) for i in range(k)]
            jax.block_until_ready(outs)
            return time.perf_counter() - t0

        run_k(2)  # warmup
        k1, k2 = bench_iters, 2 * bench_iters
        t_a = min(run_k(k1) for _ in range(2))
        t_b = min(run_k(k2) for _ in range(2))
        per_exec = (t_b - t_a) / (k2 - k1)
        _NC_CACHE["last_exec_time_ns"] = int(per_exec * 1e9)
        _NC_CACHE["bench_times_ns"] = [int(t_a * 1e9), int(t_b * 1e9)]
    results = []
    for c in range(NCORES):
        results.append({
            name: out_np[i].reshape(NCORES, *r["out_avals"][i].shape)[c]
            for i, name in enumerate(r["out_names"])})
    return results


def kernel(**inputs):
    nc = get_nc()
    in_maps = host_prep(inputs)
    if os.environ.get("KBENCH_SIM"):
        from concourse.bass_interp import MultiCoreSim
        sim = MultiCoreSim(nc, num_cores=NCORES)
        for c, core in enumerate(sim.cores.values()):
            for k, v in in_maps[c].items():
                core.tensor(k)[:] = v
        sim.simulate(check_with_hw=False)
        results = [{"out_c": np.array(core.tensor("out_c"))}
                   for core in sim.cores.values()]
        return assemble(results)
    iters = int(os.environ.get("KBENCH_ITERS", "0"))
    results = _run_hw(in_maps, bench_iters=iters)
    return assemble(results)


# revision 13
# speedup vs baseline: 1.0903x; 1.0903x over previous
"""Trainium2 Bass kernel for a Llama decoder layer (nn_MixedLlamaDecoderLayer_732).

Strategy (8-core tensor parallel, all column-parallel / all-gather based):
  - raw hidden^T replicated (bf16) to every core; ln1_w folded into Wq/Wk/Wv
    on host; per-token rsqrt factors computed locally from the core's token
    slice and exchanged via a tiny AllGather, then folded into the RoPE / V
    epilogues (exact same math as normalizing x first).
  - QKV + RoPE + causal attention head-sharded (4 Q heads / 1 KV head per
    core); attention output AllGathered per batch (4 chunks) so the
    collective hides under the next batch's compute.
  - o_proj column-parallel -> h1 column shard (fp32); ln2 stats via small
    per-pair AllReduces (4 chunks); ln2_w folded into gate/up weights;
    h2 column shard AllGathered in 4 chunks.
  - gate/up FF-sharded -> g^T AllGathered in 8 chunks (2 ff-halves x 4
    token pairs) -> down column-parallel over the gathered (reordered) g.
  - output = column shard of (h1 + mlp)^T, assembled + transposed on host.

All activations on-device are feature-major ("transposed": [features,
tokens]) so every matmul contraction dim lands on SBUF partitions.
Collectives are chunked and issued inside the producing loops so the
CC core / SDMA work overlaps PE compute instead of serializing stages.
"""

import os
import sys
from contextlib import ExitStack

os.environ.setdefault("JAX_PLATFORMS", "cpu")
if "/opt/trn_rl_repo" not in sys.path:
    sys.path.insert(0, "/opt/trn_rl_repo")

import numpy as np
import ml_dtypes

import concourse.bass as bass
import concourse.bacc as bacc
import concourse.tile as tile
from concourse import mybir

BF16 = mybir.dt.bfloat16
F32 = mybir.dt.float32
AF = mybir.ActivationFunctionType
ALU = mybir.AluOpType

NCORES = 8
B, S, HID = 4, 1024, 4096
T = B * S                      # 4096 tokens
NH, NKV, HD = 32, 8, 128
FF = 11008
EPS = 1e-6
THETA = 10000.0

QC = NH * HD // NCORES         # 512 q-cols per core (4 heads)
KC = HD                        # 128 kv-cols per core (1 kv head)
FFC = FF // NCORES             # 1376 ff per core
OC = HID // NCORES             # 512 out-cols per core
TS = T // NCORES               # 512 tokens per core
NHT = HID // 128               # 32 hid tiles
NTT = T // 512                 # 8 token tiles of 512
NPAIR = NTT // 2               # 4 token-tile pairs (= batches)
SCALE = 1.0 / float(np.sqrt(HD))

# ff tile sizes within a core's 1376 columns: 10x128 + 96, split in halves
FF_TILES = [(i * 128, 128) for i in range(10)] + [(1280, 96)]
HALVES = [FF_TILES[:6], FF_TILES[6:]]      # 768 rows | 608 rows per core
HSZ = [768, 608]
GROWS = [HSZ[0] * NCORES, HSZ[1] * NCORES]  # 6144, 4864 gathered rows
GT0 = GROWS[0] // 128                       # 48 gathered f-tiles (half 0)
GT1 = GROWS[1] // 128                       # 38 gathered f-tiles (half 1)
NFT = GT0 + GT1                             # 86


def build_nc():
    nc = bacc.Bacc("TRN2", target_bir_lowering=False, debug=False,
                   num_devices=NCORES)
    d = {}
    ein = lambda n, s, t: nc.dram_tensor(n, s, t, kind="ExternalInput")
    d["hid_c"] = ein("hid_c", [TS, HID], F32)        # own token slice (for r)
    d["hidT_f"] = ein("hidT_f", [HID, T], BF16)      # replicated raw hidden^T
    d["hidT_c"] = ein("hidT_c", [OC, T], F32)        # own hid-col slice (resid)
    d["wq_t"] = ein("wq_t", [128, NHT * QC], BF16)   # ln1-folded
    d["wk_t"] = ein("wk_t", [128, NHT * KC], BF16)
    d["wv_t"] = ein("wv_t", [128, NHT * KC], BF16)
    d["wo_t"] = ein("wo_t", [128, NHT * OC], BF16)
    d["gate_t"] = ein("gate_t", [128, NHT * FFC], BF16)  # ln2-folded
    d["up_t"] = ein("up_t", [128, NHT * FFC], BF16)
    d["down_t"] = ein("down_t", [128, NFT * OC], BF16)   # rows reordered
    d["cosT"] = ein("cosT", [128, T], BF16)
    d["sinS"] = ein("sinS", [128, T], BF16)          # sign-folded sin
    d["maskT"] = ein("maskT", [128, (S // 128) * S], BF16)
    d["ident"] = ein("ident", [128, 128], BF16)
    d["ones128"] = ein("ones128", [128, 1], BF16)
    d["ones1"] = ein("ones1", [1, 128], BF16)
    out_c = nc.dram_tensor("out_c", [OC, T], F32, kind="ExternalOutput")

    # ---- internal DRAM (collective bounce buffers) ----
    r_in = nc.dram_tensor("r_in", [1, TS], F32)
    ag_r = nc.dram_tensor("ag_r", [NCORES, TS], F32, addr_space="Shared")
    ag_at_in = [nc.dram_tensor(f"ag_at_in{b}", [QC, S], BF16)
                for b in range(B)]
    ag_at = [nc.dram_tensor(f"ag_at{b}", [NH * HD, S], BF16,
                            addr_space="Shared") for b in range(B)]
    ar_sq_in = [nc.dram_tensor(f"ar_sq_in{p}", [1, 1024], F32)
                for p in range(NPAIR)]
    ar_sq = [nc.dram_tensor(f"ar_sq{p}", [1, 1024], F32, addr_space="Shared")
             for p in range(NPAIR)]
    ag_h2_in = [nc.dram_tensor(f"ag_h2_in{p}", [OC, 1024], BF16)
                for p in range(NPAIR)]
    ag_h2 = [nc.dram_tensor(f"ag_h2{p}", [HID, 1024], BF16,
                            addr_space="Shared") for p in range(NPAIR)]
    ag_g_in = [[nc.dram_tensor(f"ag_g_in{h}_{p}", [HSZ[h], 1024], BF16)
                for p in range(NPAIR)] for h in range(2)]
    ag_g = [[nc.dram_tensor(f"ag_g{h}_{p}", [GROWS[h], 1024], BF16,
                            addr_space="Shared") for p in range(NPAIR)]
            for h in range(2)]
    h1_spill = nc.dram_tensor("h1_spill", [OC, T], F32)

    RG = [list(range(NCORES))]

    def agather(ins, outs):
        nc.gpsimd.collective_compute("AllGather", ALU.bypass,
                                     replica_groups=RG, ins=[ins], outs=[outs])

    with tile.TileContext(nc) as tc:
        with tc.tile_pool(name="consts", bufs=1) as consts:
            ident = consts.tile([128, 128], BF16)
            nc.gpsimd.dma_start(ident[:], d["ident"][:])
            ones128 = consts.tile([128, 1], BF16)
            nc.gpsimd.dma_start(ones128[:], d["ones128"][:])
            ones1 = consts.tile([1, 128], BF16)
            nc.gpsimd.dma_start(ones1[:], d["ones1"][:])
            eps128 = consts.tile([128, 1], F32)
            nc.gpsimd.memset(eps128[:], EPS)

            # ===== Stage A-lite: rms factors for own tokens + tiny AG =====
            with (
                tc.tile_pool(name="a_in", bufs=2) as a_in,
                tc.tile_pool(name="a_tmp", bufs=2) as a_tmp,
                tc.tile_pool(name="a_keep", bufs=1) as a_keep,
            ):
                rstage = a_keep.tile([128, TS // 128], F32, tag="rstage")
                ssq4 = a_keep.tile([128, TS // 128], F32, tag="ssq4")
                for p4 in range(TS // 128):
                    ht = a_in.tile([128, HID], F32, tag="hin", name=f"ht_{p4}")
                    nc.sync.dma_start(ht[:], d["hid_c"][p4 * 128:(p4 + 1) * 128, :])
                    sq = a_tmp.tile([128, HID], F32, tag="sq", name="sq")
                    nc.scalar.activation(sq[:], ht[:], AF.Square,
                                         accum_out=ssq4[:, p4:p4 + 1])
                st4 = a_tmp.tile([128, TS // 128], F32, tag="st4", name="st4")
                nc.scalar.activation(st4[:], ssq4[:], AF.Sqrt,
                                     scale=1.0 / HID, bias=eps128[:, 0:1])
                nc.vector.reciprocal(rstage[:], st4[:])
                nc.sync.dma_start(
                    r_in[:].rearrange("o (b p) -> (o p) b", p=128), rstage[:])
            agather(r_in[:], ag_r[:])

            # ===== Stage B: QKV (raw x) + r-folded RoPE (head shard) =====
            bc_es = ExitStack()
            bc_keep = bc_es.enter_context(tc.tile_pool(name="bc_keep", bufs=1))
            qT = bc_keep.tile([128, 4 * T], BF16, tag="qT")
            kT = bc_keep.tile([128, T], BF16, tag="kT")
            vS = bc_keep.tile([128, T], BF16, tag="vS")
            cosT = bc_keep.tile([128, T], BF16, tag="cosT")
            nc.gpsimd.dma_start(cosT[:], d["cosT"][:])
            sinS = bc_keep.tile([128, T], BF16, tag="sinS")
            nc.gpsimd.dma_start(sinS[:], d["sinS"][:])

            with (
                tc.tile_pool(name="b_w", bufs=1) as b_w,
                tc.tile_pool(name="b_x", bufs=2) as b_x,
                tc.tile_pool(name="b_tmp", bufs=2) as b_tmp,
                tc.tile_pool(name="b_rb", bufs=2) as b_rb,
                tc.tile_pool(name="b_ps", bufs=1, space="PSUM") as b_ps,
                tc.tile_pool(name="b_psr", bufs=1, space="PSUM") as b_psr,
            ):
                wq = b_w.tile([128, NHT * QC], BF16)
                nc.gpsimd.dma_start(wq[:], d["wq_t"][:])
                wk = b_w.tile([128, NHT * KC], BF16)
                nc.gpsimd.dma_start(wk[:], d["wk_t"][:])
                wv = b_w.tile([128, NHT * KC], BF16)
                nc.gpsimd.dma_start(wv[:], d["wv_t"][:])

                def rope_fold(dst, dst_off, ps, cs_off, rbt):
                    """dst[:, dst_off:+512] = rope(ps) * r  (r broadcast)."""
                    c_lo = cosT[0:64, cs_off:cs_off + 512]
                    c_hi = cosT[64:128, cs_off:cs_off + 512]
                    s_lo = sinS[0:64, cs_off:cs_off + 512]
                    s_hi = sinS[64:128, cs_off:cs_off + 512]
                    t1 = b_tmp.tile([128, 512], F32, tag="ro1", name="ro1")
                    nc.vector.tensor_mul(t1[0:64, :], ps[64:128, :], s_lo)
                    nc.vector.tensor_mul(t1[64:128, :], ps[0:64, :], s_hi)
                    t2 = b_tmp.tile([128, 512], F32, tag="ro2", name="ro2")
                    nc.vector.tensor_mul(t2[0:64, :], ps[0:64, :], c_lo)
                    nc.vector.tensor_mul(t2[64:128, :], ps[64:128, :], c_hi)
                    t3 = b_tmp.tile([128, 512], F32, tag="ro3", name="ro3")
                    nc.vector.tensor_add(t3[:], t1[:], t2[:])
                    nc.vector.tensor_mul(dst[:, dst_off:dst_off + 512],
                                         t3[:], rbt[:])

                for tt in range(NTT):
                    # broadcast r for this token tile: ag_r row tt
                    rrow = b_tmp.tile([1, 512], F32, tag="rrow", name=f"rr_{tt}")
                    nc.sync.dma_start(rrow[:], ag_r[tt:tt + 1, :])
                    rrowb = b_tmp.tile([1, 512], BF16, tag="rrowb", name="rrb")
                    nc.vector.tensor_copy(rrowb[:], rrow[:])
                    ps_rb = b_psr.tile([128, 512], F32, tag="tpv",
                                       name=f"ps_rb_{tt}", bufs=2)
                    nc.tensor.matmul(ps_rb[:], ones1[:], rrowb[:],
                                     start=True, stop=True)
                    rbt = b_rb.tile([128, 512], BF16, tag="rbt",
                                    name=f"rbt_{tt}")
                    nc.vector.tensor_copy(rbt[:], ps_rb[:])

                    xe = b_x.tile([128, NHT * 512], BF16, tag="xe",
                                  name=f"xe_{tt}")
                    nc.sync.dma_start(
                        xe[:].rearrange("p (a t) -> p a t", t=512),
                        d["hidT_f"][:, tt * 512:(tt + 1) * 512]
                        .rearrange("(a p) t -> p a t", p=128))
                    # separate full-K sweeps per output block -> long PE
                    # bursts; each sweep's rope drains under the next sweep
                    for qc in range(4):
                        psq = b_ps.tile([128, 512], F32, tag=f"psq{qc}",
                                        name=f"psq{qc}_{tt}")
                        for h in range(NHT):
                            nc.tensor.matmul(
                                psq[:],
                                wq[:, h * QC + qc * 128: h * QC + (qc + 1) * 128],
                                xe[:, h * 512:(h + 1) * 512],
                                start=(h == 0), stop=(h == NHT - 1))
                        rope_fold(qT, qc * T + tt * 512, psq, tt * 512, rbt)
                    psk = b_ps.tile([128, 512], F32, tag="psk", name=f"psk_{tt}")
                    for h in range(NHT):
                        nc.tensor.matmul(psk[:], wk[:, h * KC:(h + 1) * KC],
                                         xe[:, h * 512:(h + 1) * 512],
                                         start=(h == 0), stop=(h == NHT - 1))
                    rope_fold(kT, tt * 512, psk, tt * 512, rbt)
                    psv = b_ps.tile([128, 512], F32, tag="psv", name=f"psv_{tt}")
                    for h in range(NHT):
                        nc.tensor.matmul(psv[:], wv[:, h * KC:(h + 1) * KC],
                                         xe[:, h * 512:(h + 1) * 512],
                                         start=(h == 0), stop=(h == NHT - 1))
                    # V^T * r, then transpose 128-blocks into token-major vS
                    vtmp = b_tmp.tile([128, 512], BF16, tag="vtmp",
                                      name=f"vtmp_{tt}")
                    nc.vector.tensor_mul(vtmp[:], psv[:], rbt[:])
                    for s4 in range(4):
                        pvt = b_psr.tile([128, 128], BF16, tag="tpv",
                                         name=f"tpv_{tt}_{s4}", bufs=2)
                        nc.tensor.transpose(pvt[:], vtmp[:, s4 * 128:(s4 + 1) * 128],
                                            ident[:])
                        nc.vector.tensor_copy(
                            vS[:, (tt * 4 + s4) * 128:(tt * 4 + s4 + 1) * 128],
                            pvt[:])

            # ===== Stage C: causal attention, AG chunk per batch =====
            with (
                tc.tile_pool(name="c_pt", bufs=3) as c_pt,
                tc.tile_pool(name="c_keep", bufs=1) as c_keep,
                tc.tile_pool(name="c_tmp", bufs=4) as c_tmp,
                tc.tile_pool(name="c_ps", bufs=2, space="PSUM") as c_ps,
                tc.tile_pool(name="c_psd", bufs=2, space="PSUM") as c_psd,
            ):
                maskT = c_keep.tile([128, (S // 128) * S], BF16, tag="maskT")
                nc.gpsimd.dma_start(maskT[:], d["maskT"][:])
                NKT = S // 128  # 8 k tiles per batch
                for b in range(B):
                    for h in range(4):
                        pt = c_pt.tile([128, NKT * S], BF16, tag="pt",
                                       name=f"pt_{b}_{h}")
                        qoff = h * T + b * S
                        for kt in range(NKT):
                            for q2 in range(2):
                                if kt * 128 >= (q2 + 1) * 512:
                                    continue
                                pss = c_ps.tile([128, 512], F32, tag="pss",
                                                name=f"pss_{b}_{h}_{kt}_{q2}")
                                nc.tensor.matmul(
                                    pss[:],
                                    kT[:, b * S + kt * 128: b * S + (kt + 1) * 128],
                                    qT[:, qoff + q2 * 512: qoff + (q2 + 1) * 512],
                                    start=True, stop=True)
                                po = kt * S + q2 * 512
                                nc.vector.scalar_tensor_tensor(
                                    pt[:, po:po + 512], pss[:], SCALE,
                                    maskT[:, kt * S + q2 * 512: kt * S + (q2 + 1) * 512],
                                    op0=ALU.mult, op1=ALU.add)
                                nc.scalar.activation(pt[:, po:po + 512],
                                                     pt[:, po:po + 512], AF.Exp)
                        for q2 in range(2):
                            nk = min(NKT, (q2 + 1) * 4)
                            psd = c_psd.tile([1, 512], F32, tag="psd",
                                             name=f"psd_{b}_{h}_{q2}")
                            for kt in range(nk):
                                nc.tensor.matmul(
                                    psd[:], ones128[:],
                                    pt[:, kt * S + q2 * 512: kt * S + (q2 + 1) * 512],
                                    start=(kt == 0), stop=(kt == nk - 1))
                            dnr = c_tmp.tile([1, 512], F32, tag="dnr", name="dnr")
                            nc.vector.reciprocal(dnr[:], psd[:])
                            dnb = c_tmp.tile([1, 512], BF16, tag="dnb", name="dnb")
                            nc.vector.tensor_copy(dnb[:], dnr[:])
                            psr = c_psd.tile([128, 512], F32, tag="psr",
                                             name=f"psr_{b}_{h}_{q2}")
                            nc.tensor.matmul(psr[:], ones1[:], dnb[:],
                                             start=True, stop=True)
                            rb = c_tmp.tile([128, 512], BF16, tag="rb", name="rb")
                            nc.vector.tensor_copy(rb[:], psr[:])
                            psa = c_ps.tile([128, 512], F32, tag="psa",
                                            name=f"psa_{b}_{h}_{q2}")
                            for kt in range(nk):
                                nc.tensor.matmul(
                                    psa[:],
                                    vS[:, (b * 8 + kt) * 128:(b * 8 + kt + 1) * 128],
                                    pt[:, kt * S + q2 * 512: kt * S + (q2 + 1) * 512],
                                    start=(kt == 0), stop=(kt == nk - 1))
                            ao = c_tmp.tile([128, 512], BF16, tag="ao", name="ao")
                            nc.vector.tensor_mul(ao[:], psa[:], rb[:])
                            nc.sync.dma_start(
                                ag_at_in[b][h * 128:(h + 1) * 128,
                                            q2 * 512:(q2 + 1) * 512],
                                ao[:])
                    agather(ag_at_in[b][:], ag_at[b][:])
            bc_es.close()

            # E weights stream in thirds; third 0 preloads during D on the
            # gpsimd DMA ring so it never blocks D's sync-ring loads.
            THIRDS = [FF_TILES[0:4], FF_TILES[4:8], FF_TILES[8:11]]
            TSZ = [sum(fw for _, fw in q) for q in THIRDS]  # 512, 512, 352
            de_es = ExitStack()
            e_w0 = de_es.enter_context(tc.tile_pool(name="e_w0", bufs=1))

            def load_tw(pool, srcw, qi, nm):
                q_lo = THIRDS[qi][0][0]
                t = pool.tile([128, NHT * TSZ[0]], BF16, tag=nm,
                              name=f"{nm}_{qi}")
                nc.gpsimd.dma_start(
                    t[:, 0:NHT * TSZ[qi]].rearrange("p (h f) -> p h f",
                                                    f=TSZ[qi]),
                    srcw.rearrange("p (h f) -> p h f", f=FFC)
                    [:, :, q_lo:q_lo + TSZ[qi]])
                return t

            gw0 = load_tw(e_w0, d["gate_t"], 0, "gw0")
            uw0 = load_tw(e_w0, d["up_t"], 0, "uw0")

            # ===== Stage D: o_proj col-parallel + residual + chunked ln2 =====
            with (
                tc.tile_pool(name="d_w", bufs=1) as d_w,
                tc.tile_pool(name="d_x", bufs=3) as d_x,
                tc.tile_pool(name="d_tmp", bufs=2) as d_tmp,
                tc.tile_pool(name="d_h1", bufs=2) as d_h1,
                tc.tile_pool(name="d_rb", bufs=1) as d_rb,
                tc.tile_pool(name="d_ps", bufs=1, space="PSUM") as d_ps,
                tc.tile_pool(name="d_pss", bufs=2, space="PSUM") as d_pss,
            ):
                wo = d_w.tile([128, NHT * OC], BF16)
                nc.gpsimd.dma_start(wo[:], d["wo_t"][:])
                for p in range(NPAIR):
                    h1bf = d_h1.tile([128, 4 * 1024], BF16, tag="h1bf",
                                     name=f"h1bf_{p}")
                    sqst = d_h1.tile([1, 1024], F32, tag="sqst",
                                     name=f"sqst_{p}")
                    for ti in range(2):
                        tt = 2 * p + ti
                        pso = [d_ps.tile([128, 512], F32, tag=f"pso{i}",
                                         name=f"pso{i}_{tt}") for i in range(4)]
                        for a4 in range(NHT // 4):
                            at4 = d_x.tile([128, 4 * 512], BF16, tag="at",
                                           name=f"at_{tt}_{a4}")
                            nc.sync.dma_start(
                                at4[:].rearrange("p (a t) -> p a t", t=512),
                                ag_at[p][a4 * 512:(a4 + 1) * 512,
                                         ti * 512:(ti + 1) * 512]
                                .rearrange("(a p) t -> p a t", p=128))
                            for a in range(4):
                                ac = a4 * 4 + a
                                at = at4[:, a * 512:(a + 1) * 512]
                                for oc in range(4):
                                    nc.tensor.matmul(
                                        pso[oc][:],
                                        wo[:, ac * OC + oc * 128: ac * OC + (oc + 1) * 128],
                                        at, start=(ac == 0), stop=(ac == NHT - 1))
                        ps_ssq = d_pss.tile([1, 512], F32, tag="ps_ssq",
                                            name=f"ps_ssq_{tt}")
                        hT4 = d_tmp.tile([128, 4 * 512], F32, tag="hTt",
                                         name="hT4")
                        nc.sync.dma_start(
                            hT4[:].rearrange("p (a t) -> p a t", t=512),
                            d["hidT_c"][:, tt * 512:(tt + 1) * 512]
                            .rearrange("(a p) t -> p a t", p=128))
                        h1f4 = d_tmp.tile([128, 4 * 512], F32, tag="h1f",
                                          name="h1f4")
                        for oc in range(4):
                            h1f = h1f4[:, oc * 512:(oc + 1) * 512]
                            nc.vector.tensor_add(h1f, pso[oc][:],
                                                 hT4[:, oc * 512:(oc + 1) * 512])
                            hb_off = oc * 1024 + ti * 512
                            nc.vector.tensor_copy(h1bf[:, hb_off:hb_off + 512],
                                                  h1f)
                            h1sq = d_tmp.tile([128, 512], BF16, tag="h1sq",
                                              name="h1sq")
                            nc.scalar.activation(h1sq[:], h1f, AF.Square)
                            nc.tensor.matmul(ps_ssq[:], ones128[:], h1sq[:],
                                             start=(oc == 0), stop=(oc == 3))
                        nc.sync.dma_start(
                            h1_spill[:, tt * 512:(tt + 1) * 512]
                            .rearrange("(a p) t -> p a t", p=128),
                            h1f4[:].rearrange("p (a t) -> p a t", t=512))
                        nc.vector.tensor_copy(sqst[:, ti * 512:(ti + 1) * 512],
                                              ps_ssq[:])
                    nc.sync.dma_start(ar_sq_in[p][:], sqst[:])
                    nc.gpsimd.collective_compute(
                        "AllReduce", ALU.add, replica_groups=RG,
                        ins=[ar_sq_in[p][:]], outs=[ar_sq[p][:]])
                    ssqF = d_tmp.tile([1, 1024], F32, tag="u1", name=f"ssqF_{p}")
                    nc.sync.dma_start(ssqF[:], ar_sq[p][:])
                    sF = d_tmp.tile([1, 1024], F32, tag="u0", name="sF")
                    nc.scalar.activation(sF[:], ssqF[:], AF.Sqrt,
                                         scale=1.0 / HID, bias=eps128[0:1, 0:1])
                    rF = d_tmp.tile([1, 1024], F32, tag="u1b", name="rF")
                    nc.vector.reciprocal(rF[:], sF[:])
                    rFb = d_tmp.tile([1, 1024], BF16, tag="rFb", name="rFb")
                    nc.vector.tensor_copy(rFb[:], rF[:])
                    rb2 = d_rb.tile([128, 1024], BF16, tag="rb2",
                                    name=f"rb2_{p}")
                    for ti in range(2):
                        psrb = d_pss.tile([128, 512], F32, tag="psrb",
                                          name=f"psrb_{p}_{ti}")
                        nc.tensor.matmul(psrb[:], ones1[:],
                                         rFb[:, ti * 512:(ti + 1) * 512],
                                         start=True, stop=True)
                        nc.vector.tensor_copy(rb2[:, ti * 512:(ti + 1) * 512],
                                              psrb[:])
                    for ti in range(2):
                        h2t4 = d_tmp.tile([128, 4 * 512], BF16, tag="h2t",
                                          name=f"h2t4_{p}_{ti}")
                        for oc in range(4):
                            nc.vector.tensor_mul(
                                h2t4[:, oc * 512:(oc + 1) * 512],
                                h1bf[:, oc * 1024 + ti * 512: oc * 1024 + (ti + 1) * 512],
                                rb2[:, ti * 512:(ti + 1) * 512])
                        nc.sync.dma_start(
                            ag_h2_in[p][:, ti * 512:(ti + 1) * 512]
                            .rearrange("(a p) t -> p a t", p=128),
                            h2t4[:].rearrange("p (a t) -> p a t", t=512))
                    agather(ag_h2_in[p][:], ag_h2[p][:])

            # ===== Stage E: gate/up (ff shard), third-streamed weights ====
            with (
                tc.tile_pool(name="e_x", bufs=2) as e_x,
                tc.tile_pool(name="e_tmp", bufs=3) as e_tmp,
                tc.tile_pool(name="e_ps", bufs=2, space="PSUM") as e_ps,
            ):
                for qi in range(3):
                    if qi == 0:
                        gw, uw = gw0, uw0
                    else:
                        gw = load_tw(e_w0, d["gate_t"], qi, "gw0")
                        uw = load_tw(e_w0, d["up_t"], qi, "uw0")
                    q_lo = THIRDS[qi][0][0]
                    for p in range(NPAIR):
                        for ti in range(2):
                            tt = 2 * p + ti
                            h2full = e_x.tile([128, NHT * 512], BF16,
                                              tag="h2full",
                                              name=f"h2full_{qi}_{tt}")
                            nc.sync.dma_start(
                                h2full[:].rearrange("p (a t) -> p a t", t=512),
                                ag_h2[p][:, ti * 512:(ti + 1) * 512]
                                .rearrange("(a p) t -> p a t", p=128))
                            for (fo, fw) in THIRDS[qi]:
                                psg = e_ps.tile([128, 512], F32, tag="psg",
                                                name=f"psg_{tt}_{fo}")
                                psu = e_ps.tile([128, 512], F32, tag="psu",
                                                name=f"psu_{tt}_{fo}")
                                lo = fo - q_lo
                                for h in range(NHT):
                                    nc.tensor.matmul(
                                        psg[0:fw, :],
                                        gw[:, h * TSZ[qi] + lo: h * TSZ[qi] + lo + fw],
                                        h2full[:, h * 512:(h + 1) * 512],
                                        start=(h == 0), stop=(h == NHT - 1))
                                for h in range(NHT):
                                    nc.tensor.matmul(
                                        psu[0:fw, :],
                                        uw[:, h * TSZ[qi] + lo: h * TSZ[qi] + lo + fw],
                                        h2full[:, h * 512:(h + 1) * 512],
                                        start=(h == 0), stop=(h == NHT - 1))
                                gsig = e_tmp.tile([128, 512], F32, tag="gsig",
                                                  name="gsig")
                                nc.scalar.activation(gsig[0:fw, :], psg[0:fw, :],
                                                     AF.Sigmoid)
                                gsil = e_tmp.tile([128, 512], BF16, tag="gsil",
                                                  name="gsil")
                                nc.vector.tensor_mul(gsil[0:fw, :], gsig[0:fw, :],
                                                     psg[0:fw, :])
                                gt = e_tmp.tile([128, 512], BF16, tag="gt",
                                                name="gt")
                                nc.vector.tensor_mul(gt[0:fw, :], gsil[0:fw, :],
                                                     psu[0:fw, :])
                                hi = 0 if fo < HSZ[0] else 1
                                h_lo = 0 if hi == 0 else HSZ[0]
                                nc.sync.dma_start(
                                    ag_g_in[hi][p][fo - h_lo:fo - h_lo + fw,
                                                   ti * 512:(ti + 1) * 512],
                                    gt[0:fw, :])
                        if qi == 1:
                            agather(ag_g_in[0][p][:], ag_g[0][p][:])
                        elif qi == 2:
                            agather(ag_g_in[1][p][:], ag_g[1][p][:])
            de_es.close()

            # ===== Stage F: down col-parallel over gathered g + residual ====
            with (
                tc.tile_pool(name="f_w", bufs=1) as f_w,
                tc.tile_pool(name="f_x", bufs=4) as f_x,
                tc.tile_pool(name="f_tmp", bufs=2) as f_tmp,
                tc.tile_pool(name="f_ps", bufs=2, space="PSUM") as f_ps,
            ):
                FCH = [22, 22, 22, 20]
                dws, f_off = [], 0
                for ci, nfc in enumerate(FCH):
                    t = f_w.tile([128, nfc * OC], BF16, tag=f"dw{ci}",
                                 name=f"dw{ci}")
                    nc.gpsimd.dma_start(
                        t[:], d["down_t"][:, f_off * OC:(f_off + nfc) * OC])
                    dws.append((f_off, t))
                    f_off += nfc
                for tt in range(NTT):
                    p, ti = tt // 2, tt % 2
                    psd = [f_ps.tile([128, 512], F32, tag=f"psd{oc}",
                                     name=f"psd{oc}_{tt}")
                           for oc in range(4)]
                    f_base = 0
                    for hi in range(2):
                        nht = GROWS[hi] // 128
                        for f4 in range(0, nht, 4):
                            nf = min(4, nht - f4)
                            gx4 = f_x.tile([128, 4 * 512], BF16, tag="gx",
                                           name=f"gx_{tt}_{hi}_{f4}")
                            nc.sync.dma_start(
                                gx4[:, 0:nf * 512]
                                .rearrange("p (a t) -> p a t", t=512),
                                ag_g[hi][p][f4 * 128:(f4 + nf) * 128,
                                            ti * 512:(ti + 1) * 512]
                                .rearrange("(a p) t -> p a t", p=128))
                            for a in range(nf):
                                f = f_base + f4 + a
                                ci = min(f // 22, 3)
                                c_off, dwt = dws[ci]
                                fl = f - c_off
                                gx = gx4[:, a * 512:(a + 1) * 512]
                                for oc in range(4):
                                    nc.tensor.matmul(
                                        psd[oc][:],
                                        dwt[:, fl * OC + oc * 128: fl * OC + (oc + 1) * 128],
                                        gx, start=(f == 0), stop=(f == NFT - 1))
                        f_base += nht
                    h1t4 = f_tmp.tile([128, 4 * 512], F32, tag="h1t",
                                      name="h1t4")
                    nc.sync.dma_start(
                        h1t4[:].rearrange("p (a t) -> p a t", t=512),
                        h1_spill[:, tt * 512:(tt + 1) * 512]
                        .rearrange("(a p) t -> p a t", p=128))
                    ot4 = f_tmp.tile([128, 4 * 512], F32, tag="ot", name="ot4")
                    for oc in range(4):
                        nc.vector.tensor_add(ot4[:, oc * 512:(oc + 1) * 512],
                                             psd[oc][:],
                                             h1t4[:, oc * 512:(oc + 1) * 512])
                    nc.sync.dma_start(
                        out_c[:, tt * 512:(tt + 1) * 512]
                        .rearrange("(a p) t -> p a t", p=128),
                        ot4[:].rearrange("p (a t) -> p a t", t=512))

    nc.compile()
    return nc


def host_prep(inputs):
    """Build the 8 per-core input maps from full-size inputs."""
    bf = ml_dtypes.bfloat16
    hs = np.asarray(inputs["hidden_states"], np.float32)
    pos = np.asarray(inputs["position_ids"]).astype(np.int64).reshape(-1)  # [T]
    mask = np.asarray(inputs["attn_mask"], np.float32).reshape(S, S)
    ln1 = np.asarray(inputs["ln1_w"], np.float32)
    ln2 = np.asarray(inputs["ln2_w"], np.float32)
    # fold rmsnorm gains into the consuming weights (exact rewrite)
    Wq = ln1[:, None] * np.asarray(inputs["Wq"], np.float32)
    Wk = ln1[:, None] * np.asarray(inputs["Wk"], np.float32)
    Wv = ln1[:, None] * np.asarray(inputs["Wv"], np.float32)
    Wo = np.asarray(inputs["Wo"], np.float32)
    wg = ln2[:, None] * np.asarray(inputs["w_gate"], np.float32)
    wu = ln2[:, None] * np.asarray(inputs["w_up"], np.float32)
    wd = np.asarray(inputs["w_down"], np.float32)

    hsT = np.ascontiguousarray(hs.T)
    hsT_bf = hsT.astype(bf)

    invf = 1.0 / (THETA ** (np.arange(0, HD, 2, dtype=np.float32) / HD))  # [64]
    ang = pos[None, :].astype(np.float32) * np.concatenate([invf, invf])[:, None]
    cosT = np.cos(ang)                    # [128, T]
    sinT = np.sin(ang)
    sinS = sinT.copy()
    sinS[:64] *= -1.0

    maskT = np.ascontiguousarray(mask.T)  # [k, q]
    maskT_b = maskT.reshape(S // 128, 128, S).transpose(1, 0, 2).reshape(128, -1)

    def tile_w(w, cols):
        # [HID, cols] -> [128, NHT*cols] with [:, h*cols:+cols] = w[128h:+128, :]
        return np.ascontiguousarray(
            w.reshape(NHT, 128, cols).transpose(1, 0, 2).reshape(128, NHT * cols)
        ).astype(bf)

    # down rows reordered to match the gathered-g layout:
    # half0: [core0 rows 0:768, core1 rows 0:768, ...], half1: [core0 768:1376,..]
    perm = np.concatenate(
        [np.arange(c * FFC, c * FFC + HSZ[0]) for c in range(NCORES)] +
        [np.arange(c * FFC + HSZ[0], (c + 1) * FFC) for c in range(NCORES)])
    wd_r = wd[perm]

    ident = np.eye(128, dtype=np.float32).astype(bf)
    ones128 = np.ones((128, 1), np.float32).astype(bf)
    ones1 = np.ones((1, 128), np.float32).astype(bf)

    in_maps = []
    for c in range(NCORES):
        qs, ks = c * QC, c * KC
        fs, os_ = c * FFC, c * OC
        wd_cols = np.ascontiguousarray(wd_r[:, os_:os_ + OC])  # [FF, OC]
        m = {
            "hid_c": np.ascontiguousarray(hs[c * TS:(c + 1) * TS]),
            "hidT_f": hsT_bf,
            "hidT_c": np.ascontiguousarray(hsT[os_:os_ + OC]),
            "wq_t": tile_w(np.ascontiguousarray(Wq[:, qs:qs + QC]), QC),
            "wk_t": tile_w(np.ascontiguousarray(Wk[:, ks:ks + KC]), KC),
            "wv_t": tile_w(np.ascontiguousarray(Wv[:, ks:ks + KC]), KC),
            "wo_t": tile_w(np.ascontiguousarray(Wo[:, os_:os_ + OC]), OC),
            "gate_t": tile_w(np.ascontiguousarray(wg[:, fs:fs + FFC]), FFC),
            "up_t": tile_w(np.ascontiguousarray(wu[:, fs:fs + FFC]), FFC),
            "down_t": np.ascontiguousarray(
                wd_cols.reshape(NFT, 128, OC).transpose(1, 0, 2)
                .reshape(128, NFT * OC)).astype(bf),
            "cosT": cosT.astype(bf),
            "sinS": sinS.astype(bf),
            "maskT": maskT_b.astype(bf),
            "ident": ident,
            "ones128": ones128,
            "ones1": ones1,
        }
        in_maps.append(m)
    return in_maps


_NC_CACHE = {}


def get_nc():
    if "nc" not in _NC_CACHE:
        _NC_CACHE["nc"] = build_nc()
    return _NC_CACHE["nc"]


def assemble(results):
    outT = np.concatenate([results[c]["out_c"] for c in range(NCORES)], axis=0)
    return np.ascontiguousarray(outT.T.astype(np.float32))


def _get_runner():
    """Build (once) a jitted SPMD callable over the 8 neuron cores."""
    if "runner" in _NC_CACHE:
        return _NC_CACHE["runner"]
    import jax
    from jax.sharding import Mesh, PartitionSpec, NamedSharding
    from jax.experimental.shard_map import shard_map
    from concourse import bass2jax, mybir as mb
    from concourse.bass2jax import _bass_exec_p, install_neuronx_cc_hook

    nc = get_nc()
    install_neuronx_cc_hook()
    in_names, out_names, out_avals, zero_outs = [], [], [], []
    partition_name = (nc.partition_id_tensor.name
                      if nc.partition_id_tensor else None)
    for alloc in nc.m.functions[0].allocations:
        if not isinstance(alloc, mb.MemoryLocationSet):
            continue
        name = alloc.memorylocations[0].name
        if alloc.kind == "ExternalInput":
            if name != partition_name:
                in_names.append(name)
        elif alloc.kind == "ExternalOutput":
            out_names.append(name)
            shape = tuple(alloc.tensor_shape)
            dtype = mb.dt.np(alloc.dtype)
            out_avals.append(jax.core.ShapedArray(shape, dtype))
            zero_outs.append(np.zeros(shape, dtype))
    n_params = len(in_names)
    n_outs = len(out_avals)
    all_in_names = list(in_names) + list(out_names)
    if partition_name is not None:
        all_in_names.append(partition_name)

    def _body(*args):
        operands = list(args)
        if partition_name is not None:
            operands.append(bass2jax.partition_id_tensor())
        outs = _bass_exec_p.bind(
            *operands,
            out_avals=tuple(out_avals),
            in_names=tuple(all_in_names),
            out_names=tuple(out_names),
            lowering_input_output_aliases=(),
            sim_require_finite=True,
            sim_require_nnan=True,
            nc=nc,
        )
        return tuple(outs)

    devices = jax.devices()[:NCORES]
    mesh = Mesh(np.asarray(devices), ("core",))
    donate = tuple(range(n_params, n_params + n_outs))
    sharded = jax.jit(
        shard_map(_body, mesh=mesh,
                  in_specs=(PartitionSpec("core"),) * (n_params + n_outs),
                  out_specs=(PartitionSpec("core"),) * n_outs,
                  check_rep=False),
        donate_argnums=donate, keep_unused=True)
    runner = {
        "jax": jax, "sharded": sharded, "in_names": in_names,
        "out_names": out_names, "out_avals": out_avals,
        "zero_outs": zero_outs, "mesh": mesh,
        "sharding": NamedSharding(mesh, PartitionSpec("core")),
    }
    _NC_CACHE["runner"] = runner
    return runner


def _run_hw(in_maps, bench_iters=0):
    r = _get_runner()
    jax = r["jax"]
    concat_in = [
        np.concatenate([np.asarray(in_maps[c][n]) for c in range(NCORES)],
                       axis=0) for n in r["in_names"]]
    concat_zeros = [np.zeros((NCORES * z.shape[0], *z.shape[1:]), z.dtype)
                    for z in r["zero_outs"]]
    din = [jax.device_put(a, r["sharding"]) for a in concat_in]
    out = r["sharded"](*din, *[jax.device_put(z, r["sharding"])
                               for z in concat_zeros])
    jax.block_until_ready(out)
    out_np = [np.asarray(o) for o in out]
    if bench_iters:
        import time
        import jax.numpy as jnp
        zshapes = [(NCORES * z.shape[0], *z.shape[1:]) for z in r["zero_outs"]]
        zdtypes = [z.dtype for z in r["zero_outs"]]
        zfn = jax.jit(
            lambda: tuple(jnp.zeros(s, d) for s, d in zip(zshapes, zdtypes)),
            out_shardings=tuple(r["sharding"] for _ in zshapes))

        def run_k(k):
            dzs = [zfn() for _ in range(k)]
            jax.block_until_ready(dzs)
            t0 = time.perf_counter()
            outs = [r["sharded"](*din, *dzs[i]) for i in range(k)]
            jax.block_until_ready(outs)
            return time.perf_counter() - t0

        run_k(2)  # warmup
        k1, k2 = bench_iters, 2 * bench_iters
        t_a = min(run_k(k1) for _ in range(2))
        t_b = min(run_k(k2) for _ in range(2))
        per_exec = (t_b - t_a) / (k2 - k1)
        _NC_CACHE["last_exec_time_ns"] = int(per_exec * 1e9)
        _NC_CACHE["bench_times_ns"] = [int(t_a * 1e9), int(t_b * 1e9)]
    results = []
    for c in range(NCORES):
        results.append({
            name: out_np[i].reshape(NCORES, *r["out_avals"][i].shape)[c]
            for i, name in enumerate(r["out_names"])})
    return results


def kernel(**inputs):
    nc = get_nc()
    in_maps = host_prep(inputs)
    if os.environ.get("KBENCH_SIM"):
        from concourse.bass_interp import MultiCoreSim
        sim = MultiCoreSim(nc, num_cores=NCORES)
        for c, core in enumerate(sim.cores.values()):
            for k, v in in_maps[c].items():
                core.tensor(k)[:] = v
        sim.simulate(check_with_hw=False)
        results = [{"out_c": np.array(core.tensor("out_c"))}
                   for core in sim.cores.values()]
        return assemble(results)
    iters = int(os.environ.get("KBENCH_ITERS", "0"))
    results = _run_hw(in_maps, bench_iters=iters)
    return assemble(results)
